# revision 6
# baseline (speedup 1.0000x reference)
"""GroupHadamardLayer (segment_reduce) Trainium2 kernel.

The reference computes, for arbitrary group_idx:
    gathered = x[:, group_idx]                # [B, 256, 8]
    h = einsum('bng,ng->bn', gathered, gc_w)  # [B, 256]
    h = h * diag_w
    out = h @ fc_w                            # [B, 1]

This is linear in x, so it collapses to out = x @ w with
    w[group_idx[n, g]] += gc_w[n, g] * diag_w[n] * fc_w[n, 0]
(scatter-add — exact for duplicate indices too).

Device kernel: pure memory-bound matvec. x [16384, 2048] f32 (128 MiB) is
sharded by batch across 8 cores (2048 rows / 16 MiB each). Each core loads
its shard in 4 MiB chunks ([128 partitions, 4 row-groups, 2048 cols]) and,
per 128-row group, runs one fused VectorE tensor_tensor_reduce against a
partition-replicated copy of w, yielding the 128 per-row dot products.
"""

import os
import sys

sys.path.insert(0, "/opt/trn_rl_repo")

import numpy as np

from concourse import bacc, bass, tile
from concourse.bass_utils import run_bass_kernel_spmd

mybir = bass.mybir
F32 = mybir.dt.float32

B, F = 16384, 2048
N_CORES = 8
ROWS = B // N_CORES  # 2048 rows per core
P = 128
G = 4  # 128-row groups per DMA chunk -> [128, 4*2048] f32 = 4 MiB per dma
N_TILES = ROWS // P  # 16
N_CHUNKS = N_TILES // G  # 4

_NC = None
LAST_RESULT = None  # BassKernelResults of the most recent run (for test.py)


def _build_nc():
    # Bacc (not plain Bass): its finalize() runs generate_event_semaphores,
    # which splits multi-sem waits — TRN2 ISA allows 1 sync wait per inst.
    nc = bacc.Bacc("TRN2", target_bir_lowering=False, debug=False)
    x = nc.dram_tensor("x", [ROWS, F], F32, kind="ExternalInput")
    wrep = nc.dram_tensor("wrep", [P, F], F32, kind="ExternalInput")
    out = nc.dram_tensor("out", [P, N_TILES], F32, kind="ExternalOutput")

    with tile.TileContext(nc) as tc:
        with (
            tc.tile_pool(name="xp", bufs=2) as xp,
            tc.tile_pool(name="pp", bufs=3) as pp,
            tc.tile_pool(name="wp", bufs=1) as wp,
            tc.tile_pool(name="op", bufs=1) as op,
        ):
            w_t = wp.tile([P, F], F32)
            nc.sync.dma_start(w_t[:], wrep.ap())
            out_t = op.tile([P, N_TILES], F32)
            # Warm-up: absorb the w-DMA wait on DVE so later TensorTensor ops
            # carry at most one sync wait (TRN2 TT ISA slot limit).
            scrap = wp.tile([P, 1], F32)
            nc.vector.tensor_scalar_mul(scrap[:], w_t[:, 0:1], 1.0)

            # rows r = c*(G*P) + g*P + p  ->  chunk c holds [p, g, cols]
            xv = x.ap().rearrange("(c g p) n -> c p g n", g=G, p=P)
            for c in range(N_CHUNKS):
                x_t = xp.tile([P, G, F], F32)
                nc.sync.dma_start(x_t[:], xv[c])
                for g in range(G):
                    t = c * G + g
                    prod = pp.tile([P, F], F32)
                    # VectorE: prod = x_rowgroup * w
                    nc.vector.tensor_tensor(
                        out=prod[:],
                        in0=x_t[:, g, :],
                        in1=w_t[:],
                        op=mybir.AluOpType.mult,
                    )
                    # ScalarE: row dot product = sum_free(prod)
                    nc.scalar.activation(
                        out=prod[:],
                        in_=prod[:],
                        func=mybir.ActivationFunctionType.Copy,
                        accum_out=out_t[:, t : t + 1],
                    )
            nc.sync.dma_start(out.ap(), out_t[:])
    nc.finalize()
    return nc


def kernel(x, group_idx, gc_w, diag_w, fc_w):
    global _NC, LAST_RESULT
    x = np.ascontiguousarray(np.asarray(x, dtype=np.float32))
    gi = np.asarray(group_idx).astype(np.int64)
    gc_w = np.asarray(gc_w, dtype=np.float32)
    diag_w = np.asarray(diag_w, dtype=np.float32).reshape(-1)
    fc_w = np.asarray(fc_w, dtype=np.float32).reshape(-1, 1)

    # Fold everything linear into one combined weight vector (exact).
    coef = gc_w * diag_w[:, None] * fc_w  # [256, 8]
    w = np.zeros(F, dtype=np.float32)
    np.add.at(w, gi.ravel(), coef.ravel().astype(np.float32))
    wrep = np.ascontiguousarray(np.broadcast_to(w, (P, F))).astype(np.float32)

    if _NC is None:
        _NC = _build_nc()

    in_maps = [
        {"x": np.ascontiguousarray(x[i * ROWS : (i + 1) * ROWS]), "wrep": wrep}
        for i in range(N_CORES)
    ]
    trace = bool(int(os.environ.get("TRN_KERNEL_TRACE", "0")))
    LAST_RESULT = run_bass_kernel_spmd(
        _NC, in_maps, list(range(N_CORES)), trace=trace
    )
    # out[p, t] is the dot product for shard row t*128 + p
    shard_outs = [
        LAST_RESULT.results[i]["out"].T.reshape(ROWS) for i in range(N_CORES)
    ]
    return np.concatenate(shard_outs).reshape(B, 1).astype(np.float32)


# revision 7
# speedup vs baseline: 1.0873x; 1.0873x over previous
"""GroupHadamardLayer (segment_reduce) Trainium2 kernel.

The reference computes, for arbitrary group_idx:
    gathered = x[:, group_idx]                # [B, 256, 8]
    h = einsum('bng,ng->bn', gathered, gc_w)  # [B, 256]
    h = h * diag_w
    out = h @ fc_w                            # [B, 1]

This is linear in x, so it collapses to out = x @ w with
    w[group_idx[n, g]] += gc_w[n, g] * diag_w[n] * fc_w[n, 0]
(scatter-add — exact for duplicate indices too).

Device kernel: pure memory-bound matvec. x [16384, 2048] f32 (128 MiB) is
sharded by batch across 8 cores (2048 rows / 16 MiB each). Each core
streams its shard in 2 MiB chunks ([128 partitions, 2 row-groups, 2048
cols]). Per 128-row group: an elementwise multiply against the
partition-replicated w (VectorE, 1/4 of tiles on GpSimd to balance load),
then a free-dim accumulate on ScalarE (activation Copy + accum_out) giving
the 128 per-row dot products. All compute hides under the DMA stream.
"""

import os
import sys

sys.path.insert(0, "/opt/trn_rl_repo")

import numpy as np

from concourse import bacc, bass, tile
from concourse.bass_utils import run_bass_kernel_spmd

mybir = bass.mybir
F32 = mybir.dt.float32

B, F = 16384, 2048
N_CORES = 8
ROWS = B // N_CORES  # 2048 rows per core
P = 128
G = 2  # 128-row groups per DMA chunk -> [128, 2*2048] f32 = 2 MiB per dma
N_TILES = ROWS // P  # 16
N_CHUNKS = N_TILES // G  # 8

_NC = None
LAST_RESULT = None  # BassKernelResults of the most recent run (for test.py)


def _build_nc():
    # Bacc (not plain Bass): its finalize() runs generate_event_semaphores,
    # which splits multi-sem waits — TRN2 ISA allows 1 sync wait per inst.
    nc = bacc.Bacc("TRN2", target_bir_lowering=False, debug=False)
    x = nc.dram_tensor("x", [ROWS, F], F32, kind="ExternalInput")
    w = nc.dram_tensor("w", [1, F], F32, kind="ExternalInput")
    out = nc.dram_tensor("out", [P, N_TILES], F32, kind="ExternalOutput")

    with tile.TileContext(nc) as tc:
        with (
            tc.tile_pool(name="xp", bufs=3) as xp,
            tc.tile_pool(name="pp", bufs=4) as pp,
            tc.tile_pool(name="wp", bufs=1) as wp,
            tc.tile_pool(name="op", bufs=1) as op,
        ):
            # Replicate w to all partitions during the DMA (stride-0 source).
            w_t = wp.tile([P, F], F32)
            nc.sync.dma_start(w_t[:], w.ap().broadcast_to((P, F)))
            out_t = op.tile([P, N_TILES], F32)
            dummy = wp.tile([P, 1], F32)

            # rows r = c*(G*P) + g*P + p  ->  chunk c holds [p, g, cols]
            xv = x.ap().rearrange("(c g p) n -> c p g n", g=G, p=P)
            for c in range(N_CHUNKS):
                x_t = xp.tile([P, G, F], F32)
                nc.sync.dma_start(x_t[:], xv[c])
                for g in range(G):
                    t = c * G + g
                    prod = pp.tile([P, F], F32)
                    # prod = x_rowgroup * w; GpSimd takes 1/4 of the tiles so
                    # VectorE (the slower pass) stays under the DMA stream.
                    eng = nc.gpsimd if t % 4 == 3 else nc.vector
                    eng.tensor_tensor(
                        out=prod[:],
                        in0=x_t[:, g, :],
                        in1=w_t[:],
                        op=mybir.AluOpType.mult,
                    )
                    # ScalarE: row dot product = sum_free(prod). out is a
                    # stride-0 dummy — only accum_out matters.
                    nc.scalar.activation(
                        out=dummy.broadcast_to((P, F)),
                        in_=prod[:],
                        func=mybir.ActivationFunctionType.Copy,
                        accum_out=out_t[:, t : t + 1],
                    )
            nc.sync.dma_start(out.ap(), out_t[:])
    nc.finalize()
    return nc


def kernel(x, group_idx, gc_w, diag_w, fc_w):
    global _NC, LAST_RESULT
    x = np.ascontiguousarray(np.asarray(x, dtype=np.float32))
    gi = np.asarray(group_idx).astype(np.int64)
    gc_w = np.asarray(gc_w, dtype=np.float32)
    diag_w = np.asarray(diag_w, dtype=np.float32).reshape(-1)
    fc_w = np.asarray(fc_w, dtype=np.float32).reshape(-1, 1)

    # Fold everything linear into one combined weight vector (exact).
    coef = gc_w * diag_w[:, None] * fc_w  # [256, 8]
    w = np.zeros(F, dtype=np.float32)
    np.add.at(w, gi.ravel(), coef.ravel().astype(np.float32))
    w = np.ascontiguousarray(w.reshape(1, F))

    if _NC is None:
        _NC = _build_nc()

    in_maps = [
        {"x": np.ascontiguousarray(x[i * ROWS : (i + 1) * ROWS]), "w": w}
        for i in range(N_CORES)
    ]
    trace = bool(int(os.environ.get("TRN_KERNEL_TRACE", "0")))
    LAST_RESULT = run_bass_kernel_spmd(
        _NC, in_maps, list(range(N_CORES)), trace=trace
    )
    # out[p, t] is the dot product for shard row t*128 + p
    shard_outs = [
        LAST_RESULT.results[i]["out"].T.reshape(ROWS) for i in range(N_CORES)
    ]
    return np.concatenate(shard_outs).reshape(B, 1).astype(np.float32)


# revision 10
# speedup vs baseline: 1.1618x; 1.0685x over previous
"""GroupHadamardLayer (segment_reduce) Trainium2 kernel.

The reference computes, for arbitrary group_idx:
    gathered = x[:, group_idx]                # [B, 256, 8]
    h = einsum('bng,ng->bn', gathered, gc_w)  # [B, 256]
    h = h * diag_w
    out = h @ fc_w                            # [B, 1]

This is linear in x, so it collapses to out = x @ w with
    w[group_idx[n, g]] += gc_w[n, g] * diag_w[n] * fc_w[n, 0]
(scatter-add — exact for duplicate indices too).

Device kernel: pure memory-bound matvec. x [16384, 2048] f32 (128 MiB) is
sharded by batch across 8 cores (2048 rows / 16 MiB each). Each core
streams its shard in 2 MiB chunks ([128 partitions, 2 row-groups, 2048
cols]). Per 128-row group: an elementwise multiply against the
partition-replicated w (VectorE, 1/4 of tiles on GpSimd to balance load),
then a free-dim accumulate on ScalarE (activation Copy + accum_out) giving
the 128 per-row dot products. All compute hides under the DMA stream.
"""

import os
import sys

sys.path.insert(0, "/opt/trn_rl_repo")

import numpy as np

from concourse import bacc, bass, tile
from concourse.bass_utils import run_bass_kernel_spmd

mybir = bass.mybir
F32 = mybir.dt.float32

B, F = 16384, 2048
N_CORES = 8
ROWS = B // N_CORES  # 2048 rows per core
P = 128
G = 2  # 128-row groups per DMA chunk -> [128, 2*2048] f32 = 2 MiB per dma
N_TILES = ROWS // P  # 16
N_CHUNKS = N_TILES // G  # 8

_NC = None
LAST_RESULT = None  # BassKernelResults of the most recent run (for test.py)


def _build_nc():
    # Bacc (not plain Bass): its finalize() runs generate_event_semaphores,
    # which splits multi-sem waits — TRN2 ISA allows 1 sync wait per inst.
    nc = bacc.Bacc("TRN2", target_bir_lowering=False, debug=False)
    x = nc.dram_tensor("x", [ROWS, F], F32, kind="ExternalInput")
    w = nc.dram_tensor("wrep", [P, F], F32, kind="ExternalInput")
    out = nc.dram_tensor("out", [P, N_TILES], F32, kind="ExternalOutput")

    with tile.TileContext(nc) as tc:
        with (
            tc.tile_pool(name="xp", bufs=3) as xp,
            tc.tile_pool(name="pp", bufs=4) as pp,
            tc.tile_pool(name="wp", bufs=1) as wp,
            tc.tile_pool(name="op", bufs=1) as op,
        ):
            # w arrives host-replicated to all 128 partitions (1 MiB,
            # contiguous; stride-0 APs and GpSimd partition_broadcast both
            # fail on this stack).
            w_t = wp.tile([P, F], F32)
            nc.sync.dma_start(w_t[:], w.ap())
            out_t = op.tile([P, N_TILES], F32)
            dummy = wp.tile([P, 1], F32)

            # rows r = c*(G*P) + g*P + p  ->  chunk c holds [p, g, cols]
            xv = x.ap().rearrange("(c g p) n -> c p g n", g=G, p=P)
            for c in range(N_CHUNKS):
                x_t = xp.tile([P, G, F], F32)
                nc.sync.dma_start(x_t[:], xv[c])
                for g in range(G):
                    t = c * G + g
                    prod = pp.tile([P, F], F32)
                    # VectorE: prod = x_rowgroup * w. (GpSimd offload was
                    # tried and reverted: its 2-input TT contends for SBUF
                    # ports and slows concurrent DVE TTs 2-3x.)
                    nc.vector.tensor_tensor(
                        out=prod[:],
                        in0=x_t[:, g, :],
                        in1=w_t[:],
                        op=mybir.AluOpType.mult,
                    )
                    # ScalarE: row dot product = sum_free(prod). out is a
                    # stride-0 dummy — only accum_out matters.
                    nc.scalar.activation(
                        out=dummy.broadcast_to((P, F)),
                        in_=prod[:],
                        func=mybir.ActivationFunctionType.Copy,
                        accum_out=out_t[:, t : t + 1],
                    )
                if t == N_TILES // 2 - 1:
                    # First half of the outputs: DMA out early so only the
                    # last few rows' results trail the final chunk.
                    nc.sync.dma_start(
                        out.ap()[:, : N_TILES // 2], out_t[:, : N_TILES // 2]
                    )
            nc.sync.dma_start(
                out.ap()[:, N_TILES // 2 :], out_t[:, N_TILES // 2 :]
            )
    nc.finalize()
    return nc


def kernel(x, group_idx, gc_w, diag_w, fc_w):
    global _NC, LAST_RESULT
    x = np.ascontiguousarray(np.asarray(x, dtype=np.float32))
    gi = np.asarray(group_idx).astype(np.int64)
    gc_w = np.asarray(gc_w, dtype=np.float32)
    diag_w = np.asarray(diag_w, dtype=np.float32).reshape(-1)
    fc_w = np.asarray(fc_w, dtype=np.float32).reshape(-1, 1)

    # Fold everything linear into one combined weight vector (exact).
    coef = gc_w * diag_w[:, None] * fc_w  # [256, 8]
    w = np.zeros(F, dtype=np.float32)
    np.add.at(w, gi.ravel(), coef.ravel().astype(np.float32))
    wrep = np.ascontiguousarray(np.broadcast_to(w, (P, F))).astype(np.float32)

    if _NC is None:
        _NC = _build_nc()

    in_maps = [
        {"x": np.ascontiguousarray(x[i * ROWS : (i + 1) * ROWS]), "wrep": wrep}
        for i in range(N_CORES)
    ]
    trace = bool(int(os.environ.get("TRN_KERNEL_TRACE", "0")))
    LAST_RESULT = run_bass_kernel_spmd(
        _NC, in_maps, list(range(N_CORES)), trace=trace
    )
    # out[p, t] is the dot product for shard row t*128 + p
    shard_outs = [
        LAST_RESULT.results[i]["out"].T.reshape(ROWS) for i in range(N_CORES)
    ]
    return np.concatenate(shard_outs).reshape(B, 1).astype(np.float32)
